# revision 25
# baseline (speedup 1.0000x reference)
"""Mixture-of-Experts kernel for Trainium2 (8 NeuronCores), fp8 DoubleRow.

Strategy (expert-parallel, sparse dispatch — per sharding hint):
  - Host computes the tiny gate (x @ Wg + bg), takes top-2, softmaxes the
    two logits, and dispatches tokens by expert id. Core e receives its
    expert's weights and routed tokens (padded to capacity C).
  - Device computes y = gelu(x @ W1 + b1) @ W2 (ungated, un-b2'd);
    host applies gate weights (with the 1/64 W2 descale folded in),
    scatter-adds per-expert outputs, and adds the exact G @ b2 term.

Precision scheme — 3-term fp8 hi/lo residual decomposition:
  Every matmul A @ B is computed as Ah@Bh + Al@Bh + Ah@Bl where
  Xh = e4m3(X), Xl = e4m3(X - Xh). Dropping the Al@Bl term and the
  residual requantization leave ~0.25% relative error per layer
  (measured 2.0e-3 end-to-end vs the fp32 reference, vs 2e-2 budget).
  Weights are pre-scaled by sqrt(fan-in) (W1*32, W2*64) so their values
  sit in e4m3's sweet spot; the 1/32 is undone by the gelu activation's
  scale parameter, the 1/64 is folded into the host-side gate weights.

Why fp8: TensorE DoubleRow perf mode contracts K=256 (two 128-row
planes) per pass at 0.5 cycles/output-column — 4x the bf16 MAC rate.
Three fp8 terms therefore cost 0.75x the bf16 matmul time:
  PE cycles/token = (H/128 * D/256 + D/128 * H/256) * 3 * 0.5 = 384
  vs bf16's 512, i.e. ~674us at 2.4 GHz for C = max expert load (~4213;
  capacity is exact, not rounded — every padded column costs PE time).

Per-core program, token-blocked (TB=256), software-pipelined so PE never
waits on the gelu/split chain:
  mm1_block(b): for each of 32 h-tiles: 12 DoubleRow matmuls
    (3 terms x 4 double-k) -> psum; ACT gelu(scale=1/32,+b1) -> g32 f32;
    DVE copy g32 -> hh plane (fp8), DVE subtract g32-hh -> hl plane.
    h tiles are [128, 2, TB] planes laid out exactly as mm2's DoubleRow
    moving operand, so the split writes ARE the dispatch.
  mm2_block(b): for each of 8 d-slices: 48 DoubleRow matmuls
    (3 terms x 16 double-k over H) -> psum; DVE copy -> SBUF; DMA out.
  Schedule: mm1(0); { mm1(b+1); mm2(b) } — mm2(b) reads h(b) a full
  block after it was produced, and h/x pools are double-buffered.

All weights stay SBUF-resident (128 KB/partition); h never spills to
DRAM (vs the bf16 predecessor's 101 MB restream). Total DMA ~43 MB/core
(~120us of the shared DMA engines) against ~676us of PE work.
"""

import numpy as np
import ml_dtypes

B, M, D, E, TOPK = 4096, 4, 1024, 8, 2
H = 4 * D
N = B * M
P = 128
TBF = 256            # tokens per block (half a PSUM bank of fp32)
KD1 = D // 256       # 4 double-k tiles over D (mm1 contraction)
KD2 = H // 256       # 16 double-k tiles over H (mm2 contraction)
HT = H // P          # 32 h-tiles (mm1 output rows)
DS = D // P          # 8 d-slices (mm2 output rows)
SW1, SW2 = 32.0, 64.0  # weight pre-scales (= sqrt fan-in)

E4 = ml_dtypes.float8_e4m3

_BUILD_CACHE = {}


def _build(C, repeat=1):
    """Build + compile the per-core bass program for token capacity C.
    repeat>1 python-unrolls the body for steady-state timing harnesses."""
    if (C, repeat) in _BUILD_CACHE:
        return _BUILD_CACHE[(C, repeat)]

    import concourse.mybir as mybir
    import concourse.tile as tile
    from concourse import bacc
    from concourse.alu_op_type import AluOpType

    F8 = mybir.dt.float8e4
    F32 = mybir.dt.float32
    GELU = mybir.ActivationFunctionType.Gelu
    DR = mybir.MatmulPerfMode.DoubleRow

    blocks = []
    off = 0
    while off < C:
        tb = min(TBF, C - off)
        blocks.append((off, tb))
        off += tb
    NB = len(blocks)

    nc = bacc.Bacc(trn_type="TRN2", target_bir_lowering=False, debug=False)

    # side 0 = hi, side 1 = lo residual
    xq = nc.dram_tensor("xq", [2, KD1, P, 2, C], F8, kind="ExternalInput")
    w1 = nc.dram_tensor("w1", [2, KD1, P, 2, H], F8, kind="ExternalInput")
    w2 = nc.dram_tensor("w2", [2, KD2, P, 2, D], F8, kind="ExternalInput")
    b1t = nc.dram_tensor("b1t", [P, HT], F32, kind="ExternalInput")
    yT = nc.dram_tensor("yT", [DS, P, C], F32, kind="ExternalOutput")

    # matmul term order: (w side, moving side) — hi*hi, hi*lo, lo*hi
    TERMS = [(0, 0), (0, 1), (1, 0)]

    with tile.TileContext(nc) as tc:
        with (
            tc.tile_pool(name="wsb", bufs=1) as wp,
            tc.tile_pool(name="xsb", bufs=2) as xp,
            tc.tile_pool(name="hsb", bufs=2) as hp,
            tc.tile_pool(name="gsb", bufs=3) as gp,
            tc.tile_pool(name="ysb", bufs=2) as yp,
            tc.tile_pool(name="small", bufs=1) as sp,
            tc.tile_pool(name="ps", bufs=1, space="PSUM") as pp,
        ):
            for _rep in range(repeat):
                # ---- prologue DMAs -------------------------------------
                b1sb = sp.tile([P, HT], F32, tag="b1t", name="b1sb")
                nc.scalar.dma_start(b1sb, b1t.ap())

                w1sb = [[None] * KD1 for _ in range(2)]
                w2sb = [[None] * KD2 for _ in range(2)]
                for side in range(2):
                    for k in range(KD1):
                        w1sb[side][k] = wp.tile(
                            [P, 2, H], F8, tag=f"w1_{side}_{k}",
                            name=f"w1_{side}_{k}")
                    for k in range(KD2):
                        w2sb[side][k] = wp.tile(
                            [P, 2, D], F8, tag=f"w2_{side}_{k}",
                            name=f"w2_{side}_{k}")

                def load_x(b):
                    """Allocate + DMA x tiles for block b on the (otherwise
                    idle) gpsimd queue. Returns xt[side][dk]."""
                    boff, btb = blocks[b]
                    xt = [[None] * KD1 for _ in range(2)]
                    for side in range(2):
                        for k in range(KD1):
                            t = xp.tile([P, 2, TBF], F8,
                                        tag=f"x{side}{k}", name=f"x{side}{k}")
                            nc.gpsimd.dma_start(
                                t[:, :, 0:btb],
                                xq[side][k][:, :, boff:boff + btb])
                            xt[side][k] = t
                    return xt

                # critical first loads: w1 hi chunk 0 on sync, w1 lo chunk 0
                # on scalar (PE needs the lo side only 0.4us later), x(0) and
                # x(1) on gpsimd (the opening block pair consumes both).
                xtiles = [None] * NB
                xtiles[0] = load_x(0)
                if NB > 1:
                    xtiles[1] = load_x(1)
                for k in range(KD1):
                    nc.sync.dma_start(w1sb[0][k][:, :, 0:1024],
                                      w1[0][k][:, :, 0:1024])
                for k in range(KD1):
                    nc.scalar.dma_start(w1sb[1][k][:, :, 0:1024],
                                        w1[1][k][:, :, 0:1024])
                if _rep == 0:
                    # preload the Gelu ACT table while prologue DMAs run
                    warm = sp.tile([P, 1], F32, tag="warm", name="warm")
                    nc.scalar.activation(warm, b1sb[:, 0:1], GELU)
                # rest of w1 (chunks 1-3), then w2 (hi/lo interleaved along
                # k, matching mm2's consumption order), all on sync
                for g in range(1, 4):
                    for side in range(2):
                        for k in range(KD1):
                            nc.sync.dma_start(
                                w1sb[side][k][:, :, 1024 * g:1024 * (g + 1)],
                                w1[side][k][:, :, 1024 * g:1024 * (g + 1)])
                for k in range(KD2):
                    for side in range(2):
                        nc.sync.dma_start(w2sb[side][k], w2[side][k])

                htiles = [None] * NB

                def mm1_blocks(bs):
                    """mm1 for blocks bs, ht-interleaved: 12 DoubleRow
                    matmuls -> gelu -> hh/hl split per (ht, block). The
                    opening pair [0, 1] runs interleaved so the w1 stream
                    (8.4 MB at ~360 GB/s) is consumed over two blocks' PE
                    work instead of starving block 0."""
                    hs = {}
                    for b in bs:
                        hh = [hp.tile([P, 2, TBF], F8, tag=f"hh{d}",
                                      name=f"hh{d}") for d in range(KD2)]
                        hl = [hp.tile([P, 2, TBF], F8, tag=f"hl{d}",
                                      name=f"hl{d}") for d in range(KD2)]
                        htiles[b] = (hh, hl)
                        hs[b] = (hh, hl)
                    seq = 0
                    for ht in range(HT):
                        for b in bs:
                            boff, btb = blocks[b]
                            xt = xtiles[b]
                            hh, hl = hs[b]
                            ps = pp.tile([P, 512], F32, tag=f"m1_{seq % 5}",
                                         name=f"m1_{seq % 5}")
                            seq += 1
                            nmm = len(TERMS) * KD1
                            i = 0
                            for ws, xs in TERMS:
                                for k in range(KD1):
                                    nc.tensor.matmul(
                                        ps[:, 0:btb],
                                        w1sb[ws][k][:, :, P * ht:P * (ht + 1)],
                                        xt[xs][k][:, :, 0:btb],
                                        start=(i == 0), stop=(i == nmm - 1),
                                        perf_mode=DR)
                                    i += 1
                            g32 = gp.tile([P, TBF], F32, tag="g32",
                                          name="g32")
                            nc.scalar.activation(
                                g32[:, 0:btb], ps[:, 0:btb], GELU,
                                bias=b1sb[:, ht:ht + 1], scale=1.0 / SW1)
                            d2, pl = ht // 2, ht % 2
                            nc.vector.tensor_copy(
                                hh[d2][:, pl, 0:btb], g32[:, 0:btb])
                            nc.vector.tensor_tensor(
                                hl[d2][:, pl, 0:btb], g32[:, 0:btb],
                                hh[d2][:, pl, 0:btb], op=AluOpType.subtract)

                def mm2_block(b):
                    boff, btb = blocks[b]
                    # prefetch x for the mm1 that follows this mm2; its DMA
                    # also carries the WAR wait on block b's readers (done)
                    if b + 2 < NB and xtiles[b + 2] is None:
                        xtiles[b + 2] = load_x(b + 2)
                    hh, hl = htiles[b]
                    hsides = (hh, hl)
                    for ds in range(DS):
                        ps = pp.tile([P, 512], F32, tag=f"m2_{ds % 3}",
                                     name=f"m2_{ds % 3}")
                        nmm = len(TERMS) * KD2
                        i = 0
                        for ws, hs in TERMS:
                            for k in range(KD2):
                                nc.tensor.matmul(
                                    ps[:, 0:btb],
                                    w2sb[ws][k][:, :, P * ds:P * (ds + 1)],
                                    hsides[hs][k][:, :, 0:btb],
                                    start=(i == 0), stop=(i == nmm - 1),
                                    perf_mode=DR)
                                i += 1
                        ysb = yp.tile([P, TBF], F32, tag=f"y{ds % 2}",
                                      name=f"y{ds % 2}")
                        nc.vector.tensor_copy(ysb[:, 0:btb], ps[:, 0:btb])
                        nc.sync.dma_start(
                            yT[ds][:, boff:boff + btb], ysb[:, 0:btb])

                # schedule: mm1[0,1 interleaved]; mm2(0); mm1(2); mm2(1);
                # ... — every mm1(b+2) follows mm2(b), so its h-buffer WAR
                # (bufs=2) is already satisfied when the split writes land
                mm1_blocks(list(range(min(2, NB))))
                for b in range(NB):
                    mm2_block(b)
                    if b + 2 < NB:
                        mm1_blocks([b + 2])

    nc.compile()
    _BUILD_CACHE[(C, repeat)] = nc
    return nc


def _route(xf, Wg, bg):
    """Top-2 gating on host. Returns (idx, gate) per expert and dense G."""
    logits = xf @ Wg + bg                      # [N, E] f32
    n = logits.shape[0]
    ar = np.arange(n)
    i1 = np.argmax(logits, axis=1)
    v1 = logits[ar, i1]
    masked = logits.copy()
    masked[ar, i1] = -np.inf
    i2 = np.argmax(masked, axis=1)
    v2 = masked[ar, i2]
    e2 = np.exp(v2 - v1)
    wt1 = 1.0 / (1.0 + e2)
    wt2 = e2 / (1.0 + e2)
    G = np.zeros_like(logits)
    G[ar, i1] = wt1
    G[ar, i2] = wt2
    idxs, gates = [], []
    for e in range(E):
        idx = np.nonzero((i1 == e) | (i2 == e))[0]
        idxs.append(idx)
        gates.append(G[idx, e].astype(np.float32))
    return idxs, gates, G.astype(np.float32)


def _split8(a):
    """f32 array -> (hi, lo) e4m3 pair with hi + lo ~= a."""
    hi = a.astype(E4)
    lo = (a - hi.astype(np.float32)).astype(E4)
    return hi, lo


def _pack_k(a, kd):
    """[256*kd, F] -> [kd, 128, 2, F] DoubleRow layout (row = 256k+128i+p)."""
    f = a.shape[1]
    return np.ascontiguousarray(
        a.reshape(kd, 2, P, f).transpose(0, 2, 1, 3))


def kernel(_trace=False, **inputs):
    x = np.asarray(inputs["x"], dtype=np.float32)
    Wg = np.asarray(inputs["Wg"], dtype=np.float32)
    bg = np.asarray(inputs["bg"], dtype=np.float32)
    W1 = np.asarray(inputs["W1"], dtype=np.float32)
    b1 = np.asarray(inputs["b1"], dtype=np.float32)
    W2 = np.asarray(inputs["W2"], dtype=np.float32)
    b2 = np.asarray(inputs["b2"], dtype=np.float32)

    Bn, Mn, Dn = x.shape
    n = Bn * Mn
    xf = x.reshape(n, Dn)

    idxs, gates, G = _route(xf, Wg, bg)

    C = max(max(len(i) for i in idxs), P)

    in_maps = []
    for e in range(E):
        ne = len(idxs[e])
        xe = np.zeros((C, Dn), dtype=np.float32)
        xe[:ne] = xf[idxs[e]]
        xh, xl = _split8(xe.T)                 # [D, C] each
        w1h, w1l = _split8(W1[e] * SW1)        # [D, H]
        w2h, w2l = _split8(W2[e] * SW2)        # [H, D]
        in_maps.append({
            "xq": np.stack([_pack_k(xh, KD1), _pack_k(xl, KD1)]),
            "w1": np.stack([_pack_k(w1h, KD1), _pack_k(w1l, KD1)]),
            "w2": np.stack([_pack_k(w2h, KD2), _pack_k(w2l, KD2)]),
            "b1t": np.ascontiguousarray(b1[e].reshape(HT, P).T),
        })

    nc = _build(C)

    from concourse.bass_utils import run_bass_kernel_spmd
    res = run_bass_kernel_spmd(
        nc, in_maps, core_ids=list(range(E)), trace=_trace
    )

    out = G @ b2                               # gate-weighted b2, exact
    for e in range(E):
        ne = len(idxs[e])
        # yT is [dslice, dp, tok] with D-index = dslice*128 + dp; values
        # are scaled by SW2 (folded into the gate weights below)
        ye = res.results[e]["yT"].reshape(Dn, C)[:, :ne].T
        out[idxs[e]] += (gates[e] * (1.0 / SW2))[:, None] * ye

    if _trace:
        return out.reshape(Bn, Mn, Dn), res
    return out.reshape(Bn, Mn, Dn)


# revision 32
# speedup vs baseline: 1.0129x; 1.0129x over previous
"""Mixture-of-Experts kernel for Trainium2 (8 NeuronCores), fp8 DoubleRow.

Strategy (expert-parallel, sparse dispatch — per sharding hint):
  - Host computes the tiny gate (x @ Wg + bg), takes top-2, softmaxes the
    two logits, and dispatches tokens by expert id. Core e receives its
    expert's weights and routed tokens (padded to capacity C).
  - Device computes y = gelu(x @ W1 + b1) @ W2 (ungated, un-b2'd);
    host applies gate weights (with the 1/64 W2 descale folded in),
    scatter-adds per-expert outputs, and adds the exact G @ b2 term.

Precision scheme — 3-term fp8 hi/lo residual decomposition:
  Every matmul A @ B is computed as Ah@Bh + Al@Bh + Ah@Bl where
  Xh = e4m3(X), Xl = e4m3(X - Xh). Dropping the Al@Bl term and the
  residual requantization leave ~0.25% relative error per layer
  (measured 2.0e-3 end-to-end vs the fp32 reference, vs 2e-2 budget).
  Weights are pre-scaled by sqrt(fan-in) (W1*32, W2*64) so their values
  sit in e4m3's sweet spot; the 1/32 is undone by the gelu activation's
  scale parameter, the 1/64 is folded into the host-side gate weights.

Why fp8: TensorE DoubleRow perf mode contracts K=256 (two 128-row
planes) per pass at 0.5 cycles/output-column — 4x the bf16 MAC rate.
Three fp8 terms therefore cost 0.75x the bf16 matmul time:
  PE cycles/token = (H/128 * D/256 + D/128 * H/256) * 3 * 0.5 = 384
  vs bf16's 512, i.e. ~674us at 2.4 GHz for C = max expert load (~4213;
  capacity is exact, not rounded — every padded column costs PE time).

Per-core program, token-blocked (TB=256), software-pipelined so PE never
waits on the gelu/split chain:
  mm1_block(b): for each of 32 h-tiles: 12 DoubleRow matmuls
    (3 terms x 4 double-k) -> psum; ACT gelu(scale=1/32,+b1) -> g32 f32;
    DVE copy g32 -> hh plane (fp8), DVE subtract g32-hh -> hl plane.
    h tiles are [128, 2, TB] planes laid out exactly as mm2's DoubleRow
    moving operand, so the split writes ARE the dispatch.
  mm2_block(b): for each of 8 d-slices: 48 DoubleRow matmuls
    (3 terms x 16 double-k over H) -> psum; DVE copy -> SBUF; DMA out.
  Schedule: mm1(0); { mm1(b+1); mm2(b) } — mm2(b) reads h(b) a full
  block after it was produced, and h/x pools are double-buffered.

All weights stay SBUF-resident (128 KB/partition); h never spills to
DRAM (vs the bf16 predecessor's 101 MB restream). Total DMA ~43 MB/core
(~120us of the shared DMA engines) against ~676us of PE work.
"""

import numpy as np
import ml_dtypes

B, M, D, E, TOPK = 4096, 4, 1024, 8, 2
H = 4 * D
N = B * M
P = 128
TBF = 256            # tokens per block (half a PSUM bank of fp32)
KD1 = D // 256       # 4 double-k tiles over D (mm1 contraction)
KD2 = H // 256       # 16 double-k tiles over H (mm2 contraction)
HT = H // P          # 32 h-tiles (mm1 output rows)
DS = D // P          # 8 d-slices (mm2 output rows)
SW1, SW2 = 32.0, 64.0  # weight pre-scales (= sqrt fan-in)

E4 = ml_dtypes.float8_e4m3

_BUILD_CACHE = {}


def _build(C, repeat=1):
    """Build + compile the per-core bass program for token capacity C.
    repeat>1 python-unrolls the body for steady-state timing harnesses."""
    if (C, repeat) in _BUILD_CACHE:
        return _BUILD_CACHE[(C, repeat)]

    import concourse.mybir as mybir
    import concourse.tile as tile
    from concourse import bacc
    from concourse.alu_op_type import AluOpType

    F8 = mybir.dt.float8e4
    F32 = mybir.dt.float32
    GELU = mybir.ActivationFunctionType.Gelu
    DR = mybir.MatmulPerfMode.DoubleRow

    blocks = []
    off = 0
    while off < C:
        tb = min(TBF, C - off)
        blocks.append((off, tb))
        off += tb
    NB = len(blocks)

    nc = bacc.Bacc(trn_type="TRN2", target_bir_lowering=False, debug=False)

    # side 0 = hi, side 1 = lo residual; partition-major layouts so one
    # dma_start can carry a whole side (in/out walk orders match)
    xq = nc.dram_tensor("xq", [2, P, KD1, 2, C], F8, kind="ExternalInput")
    w1 = nc.dram_tensor("w1", [2, P, KD1, 2, H], F8, kind="ExternalInput")
    w2 = nc.dram_tensor("w2", [2, P, KD2, 2, D], F8, kind="ExternalInput")
    b1t = nc.dram_tensor("b1t", [P, HT], F32, kind="ExternalInput")
    yT = nc.dram_tensor("yT", [DS, P, C], F32, kind="ExternalOutput")

    # matmul term order: (w side, moving side) — hi*hi, hi*lo, lo*hi
    TERMS = [(0, 0), (0, 1), (1, 0)]

    with tile.TileContext(nc) as tc:
        with (
            tc.tile_pool(name="wsb", bufs=1) as wp,
            tc.tile_pool(name="xsb", bufs=2) as xp,
            tc.tile_pool(name="hsb", bufs=2) as hp,
            tc.tile_pool(name="gsb", bufs=3) as gp,
            tc.tile_pool(name="ysb", bufs=2) as yp,
            tc.tile_pool(name="small", bufs=1) as sp,
            tc.tile_pool(name="ps", bufs=1, space="PSUM") as pp,
        ):
            for _rep in range(repeat):
                # ---- prologue DMAs -------------------------------------
                b1sb = sp.tile([P, HT], F32, tag="b1t", name="b1sb")

                w1sb = [wp.tile([P, KD1, 2, H], F8, tag=f"w1_{s}",
                                name=f"w1_{s}") for s in range(2)]
                w2sb = [wp.tile([P, KD2, 2, D], F8, tag=f"w2_{s}",
                                name=f"w2_{s}") for s in range(2)]

                def load_x(b):
                    """Allocate + DMA x tiles for block b: ONE dma_start per
                    side (descriptors are cheap, SWDGE's ~1us fixed cost per
                    dma_start is not) on the otherwise idle gpsimd queue.
                    Returns per-side (tile, column offset)."""
                    boff, btb = blocks[b]
                    xt = []
                    for side in range(2):
                        t = xp.tile([P, KD1, 2, TBF], F8,
                                    tag=f"x{side}", name=f"x{side}")
                        nc.gpsimd.dma_start(
                            t[:, :, :, 0:btb],
                            xq[side][:, :, :, boff:boff + btb])
                        xt.append((t, 0))
                    return xt

                # critical first loads: w1 hi chunk 0 per-dk on sync (the
                # first matmul waits only one 256 KB tile), w1 lo chunk 0 on
                # scalar, x for the whole opening pair on gpsimd as one
                # two-block-wide DMA per side.
                xtiles = [None] * NB
                pw = blocks[1][1] + TBF if NB > 1 else blocks[0][1]
                xpair = []
                for side in range(2):
                    t = xp.tile([P, KD1, 2, 2 * TBF], F8,
                                tag=f"xp{side}", name=f"xp{side}")
                    nc.gpsimd.dma_start(t[:, :, :, 0:pw],
                                        xq[side][:, :, :, 0:pw])
                    xpair.append(t)
                xtiles[0] = [(xpair[0], 0), (xpair[1], 0)]
                if NB > 1:
                    xtiles[1] = [(xpair[0], TBF), (xpair[1], TBF)]
                for k in range(KD1):
                    nc.sync.dma_start(w1sb[0][:, k, :, 0:1024],
                                      w1[0][:, k, :, 0:1024])
                for k in range(KD1):
                    nc.scalar.dma_start(w1sb[1][:, k, :, 0:1024],
                                        w1[1][:, k, :, 0:1024])
                nc.scalar.dma_start(b1sb, b1t.ap())
                if _rep == 0:
                    # preload the Gelu ACT table while prologue DMAs run
                    warm = sp.tile([P, 1], F32, tag="warm", name="warm")
                    nc.scalar.activation(warm, b1sb[:, 0:1], GELU)
                # rest of w1 (chunks 1-3) as one DMA per (side, chunk), then
                # w2 (hi/lo interleaved along k, mm2's consumption order)
                for g in range(1, 4):
                    for side in range(2):
                        nc.sync.dma_start(
                            w1sb[side][:, :, :, 1024 * g:1024 * (g + 1)],
                            w1[side][:, :, :, 1024 * g:1024 * (g + 1)])
                for k in range(KD2):
                    for side in range(2):
                        nc.sync.dma_start(w2sb[side][:, k, :, :],
                                          w2[side][:, k, :, :])

                htiles = [None] * NB

                def mm1_seq(groups):
                    """mm1 over an explicit (block, ht) sequence: 12
                    DoubleRow matmuls -> gelu -> hh/hl split per group.
                    h tiles must already be allocated in htiles[b]."""
                    for seq, (b, ht) in enumerate(groups):
                        boff, btb = blocks[b]
                        xt = xtiles[b]
                        hh, hl = htiles[b]
                        ps = pp.tile([P, 512], F32, tag=f"m1_{seq % 5}",
                                     name=f"m1_{seq % 5}")
                        nmm = len(TERMS) * KD1
                        i = 0
                        for ws, xs in TERMS:
                            xtile, xoff = xt[xs]
                            for k in range(KD1):
                                nc.tensor.matmul(
                                    ps[:, 0:btb],
                                    w1sb[ws][:, k, :, P * ht:P * (ht + 1)],
                                    xtile[:, k, :, xoff:xoff + btb],
                                    start=(i == 0), stop=(i == nmm - 1),
                                    perf_mode=DR)
                                i += 1
                        g32 = gp.tile([P, TBF], F32, tag="g32", name="g32")
                        nc.scalar.activation(
                            g32[:, 0:btb], ps[:, 0:btb], GELU,
                            bias=b1sb[:, ht:ht + 1], scale=1.0 / SW1)
                        d2, pl = ht // 2, ht % 2
                        nc.vector.tensor_copy(
                            hh[d2][:, pl, 0:btb], g32[:, 0:btb])
                        nc.vector.tensor_tensor(
                            hl[d2][:, pl, 0:btb], g32[:, 0:btb],
                            hh[d2][:, pl, 0:btb], op=AluOpType.subtract)

                def alloc_h(b):
                    hh = [hp.tile([P, 2, TBF], F8, tag=f"hh{d}",
                                  name=f"hh{d}") for d in range(KD2)]
                    hl = [hp.tile([P, 2, TBF], F8, tag=f"hl{d}",
                                  name=f"hl{d}") for d in range(KD2)]
                    htiles[b] = (hh, hl)

                def mm2_block(b):
                    boff, btb = blocks[b]
                    # prefetch x for the mm1 that follows this mm2; its DMA
                    # also carries the WAR wait on block b's readers (done)
                    if b + 2 < NB and xtiles[b + 2] is None:
                        xtiles[b + 2] = load_x(b + 2)
                    hh, hl = htiles[b]
                    hsides = (hh, hl)
                    for ds in range(DS):
                        ps = pp.tile([P, 512], F32, tag=f"m2_{ds % 3}",
                                     name=f"m2_{ds % 3}")
                        nmm = len(TERMS) * KD2
                        i = 0
                        for ws, hs in TERMS:
                            for k in range(KD2):
                                nc.tensor.matmul(
                                    ps[:, 0:btb],
                                    w2sb[ws][:, k, :, P * ds:P * (ds + 1)],
                                    hsides[hs][k][:, :, 0:btb],
                                    start=(i == 0), stop=(i == nmm - 1),
                                    perf_mode=DR)
                                i += 1
                        ysb = yp.tile([P, TBF], F32, tag=f"y{ds % 2}",
                                      name=f"y{ds % 2}")
                        nc.vector.tensor_copy(ysb[:, 0:btb], ps[:, 0:btb])
                        nc.sync.dma_start(
                            yT[ds][:, boff:boff + btb], ysb[:, 0:btb])

                # Opening pair, ht-asymmetric: [b0 ht0-7][b1 ht0-7]
                # [b0/b1 interleaved ht8-31]. The start needs only x(0) and
                # w1 chunk 0 (as a solo block would), while chunks 1-3 are
                # consumed at two blocks' PE work per chunk, hiding the
                # 6.3 MB tail of the w1 stream entirely. Then steady state:
                # mm2(b); mm1(b+2) — every mm1(b+2) follows mm2(b), so its
                # h-buffer WAR (bufs=2) is already satisfied.
                if NB > 1:
                    alloc_h(0)
                    alloc_h(1)
                    pair = ([(0, h) for h in range(8)] +
                            [(1, h) for h in range(8)])
                    for h in range(8, HT):
                        pair += [(0, h), (1, h)]
                    mm1_seq(pair)
                else:
                    alloc_h(0)
                    mm1_seq([(0, h) for h in range(HT)])
                for b in range(NB):
                    mm2_block(b)
                    if b + 2 < NB:
                        alloc_h(b + 2)
                        mm1_seq([(b + 2, h) for h in range(HT)])

    nc.compile()
    _BUILD_CACHE[(C, repeat)] = nc
    return nc


def _route(xf, Wg, bg):
    """Top-2 gating on host. Returns (idx, gate) per expert and dense G."""
    logits = xf @ Wg + bg                      # [N, E] f32
    n = logits.shape[0]
    ar = np.arange(n)
    i1 = np.argmax(logits, axis=1)
    v1 = logits[ar, i1]
    masked = logits.copy()
    masked[ar, i1] = -np.inf
    i2 = np.argmax(masked, axis=1)
    v2 = masked[ar, i2]
    e2 = np.exp(v2 - v1)
    wt1 = 1.0 / (1.0 + e2)
    wt2 = e2 / (1.0 + e2)
    G = np.zeros_like(logits)
    G[ar, i1] = wt1
    G[ar, i2] = wt2
    idxs, gates = [], []
    for e in range(E):
        idx = np.nonzero((i1 == e) | (i2 == e))[0]
        idxs.append(idx)
        gates.append(G[idx, e].astype(np.float32))
    return idxs, gates, G.astype(np.float32)


def _split8(a):
    """f32 array -> (hi, lo) e4m3 pair with hi + lo ~= a."""
    hi = a.astype(E4)
    lo = (a - hi.astype(np.float32)).astype(E4)
    return hi, lo


def _pack_k(a, kd):
    """[256*kd, F] -> [128, kd, 2, F] partition-major DoubleRow layout
    (contraction row = 256k + 128i + p)."""
    f = a.shape[1]
    return np.ascontiguousarray(
        a.reshape(kd, 2, P, f).transpose(2, 0, 1, 3))


def kernel(_trace=False, **inputs):
    x = np.asarray(inputs["x"], dtype=np.float32)
    Wg = np.asarray(inputs["Wg"], dtype=np.float32)
    bg = np.asarray(inputs["bg"], dtype=np.float32)
    W1 = np.asarray(inputs["W1"], dtype=np.float32)
    b1 = np.asarray(inputs["b1"], dtype=np.float32)
    W2 = np.asarray(inputs["W2"], dtype=np.float32)
    b2 = np.asarray(inputs["b2"], dtype=np.float32)

    Bn, Mn, Dn = x.shape
    n = Bn * Mn
    xf = x.reshape(n, Dn)

    idxs, gates, G = _route(xf, Wg, bg)

    C = max(max(len(i) for i in idxs), P)

    in_maps = []
    for e in range(E):
        ne = len(idxs[e])
        xe = np.zeros((C, Dn), dtype=np.float32)
        xe[:ne] = xf[idxs[e]]
        xh, xl = _split8(xe.T)                 # [D, C] each
        w1h, w1l = _split8(W1[e] * SW1)        # [D, H]
        w2h, w2l = _split8(W2[e] * SW2)        # [H, D]
        in_maps.append({
            "xq": np.stack([_pack_k(xh, KD1), _pack_k(xl, KD1)]),
            "w1": np.stack([_pack_k(w1h, KD1), _pack_k(w1l, KD1)]),
            "w2": np.stack([_pack_k(w2h, KD2), _pack_k(w2l, KD2)]),
            "b1t": np.ascontiguousarray(b1[e].reshape(HT, P).T),
        })

    nc = _build(C)

    from concourse.bass_utils import run_bass_kernel_spmd
    res = run_bass_kernel_spmd(
        nc, in_maps, core_ids=list(range(E)), trace=_trace
    )

    out = G @ b2                               # gate-weighted b2, exact
    for e in range(E):
        ne = len(idxs[e])
        # yT is [dslice, dp, tok] with D-index = dslice*128 + dp; values
        # are scaled by SW2 (folded into the gate weights below)
        ye = res.results[e]["yT"].reshape(Dn, C)[:, :ne].T
        out[idxs[e]] += (gates[e] * (1.0 / SW2))[:, None] * ye

    if _trace:
        return out.reshape(Bn, Mn, Dn), res
    return out.reshape(Bn, Mn, Dn)
